# revision 2
# baseline (speedup 1.0000x reference)
"""CapsLayer2D dynamic-routing kernel for 8x TRN2 NeuronCores.

Problem (hardcoded shapes):
  inputs: [B=16, R=8, C=8, I=128, DIN=16] fp32
  W:      [K=32, I=128, DIN=16, DOUT=16] fp32
  out:    [B, R, C, K, DOUT] fp32

Math (reference does 3-round dynamic routing). Closed form (verified vs
reference to ~6e-6 rel):
  U[p,k]    = res[p,k,:,:]  (I x O per position p=(b,r,c) and k)
  s0        = mean_i U_i
  v0        = squash(s0)
  t_a = U v0        ; m_a = U^T t_a ; s1 = s0 + m_a
  v1 = squash(s1)   ; vs = v0 + v1
  t_b = U vs        ; m_b = U^T t_b ; s2 = s0 + m_b
  out = squash(s2)

Sharding: batch across 8 cores (2 batches = 128 positions per core), W
replicated. No collectives.

Per-core on-device plan:
  d padded 16->32 so each input-capsule i owns a 32-aligned partition
  block (matmul operand partition base must be a multiple of 32).
  Xt  [(i,d32) chunk c of 128 rows, p]          bf16, 32 chunks
  W_r [(i,d32) chunk c of 128 rows, (k,o)=512]  bf16, 32 chunks
  Everything after that is k-group-local (4 groups of 8 output caps):
  group g: s0_g via 32 full-depth matmuls; res_g [p, (k8,i,o)] bf16 via
  128 per-i matmuls; routing passes on DVE; out cols [g*128,(g+1)*128).
  Groups pipeline: PE produces group g+1 while DVE routes group g.
"""

import sys

import numpy as np

sys.path.insert(0, "/opt/trn_rl_repo")

import ml_dtypes  # noqa: E402

P, I, D, K, O = 128, 128, 16, 32, 16
D2 = 32  # padded d
ID, KO, KI = I * D, K * O, K * I  # 2048, 512, 4096
KC = 8  # k-group size
NG = K // KC  # 4 groups
GW = KC * O  # 128 group output width
N_CORES = 8
EPS = 1e-7

_PROGRAM = None


def _build_program():
    from contextlib import ExitStack

    import concourse.bass as bass
    import concourse.tile as tile
    from concourse import bacc, mybir

    F32 = mybir.dt.float32
    BF16 = mybir.dt.float16  # fp16: same speed class as bf16, 8x finer mantissa
    MULT = mybir.AluOpType.mult
    ADD = mybir.AluOpType.add
    X = mybir.AxisListType.X
    SQRT = mybir.ActivationFunctionType.Sqrt

    # Bacc (not raw Bass): its compile() runs generate_event_semaphores,
    # which splits multi-sem waits (TRN2 allows 1 wait per instruction).
    nc = bacc.Bacc("TRN2", target_bir_lowering=False, debug=False)

    x_d = nc.dram_tensor("x", [P, ID], F32, kind="ExternalInput").ap()
    w_d = nc.dram_tensor("w", [K, I * D * O], F32, kind="ExternalInput").ap()
    e128_d = nc.dram_tensor("e128", [128, 128], BF16, kind="ExternalInput").ap()
    out_d = nc.dram_tensor("out", [P, KO], F32, kind="ExternalOutput").ap()

    with ExitStack() as ctx:
        tc = ctx.enter_context(tile.TileContext(nc))

        pp_t = ctx.enter_context(tc.tile_pool(name="pp_t", bufs=2, space="PSUM"))
        pp_w = ctx.enter_context(tc.tile_pool(name="pp_w", bufs=2, space="PSUM"))
        pp_s = ctx.enter_context(tc.tile_pool(name="pp_s", bufs=1, space="PSUM"))
        pp_r = ctx.enter_context(tc.tile_pool(name="pp_r", bufs=3, space="PSUM"))

        rp = ctx.enter_context(tc.tile_pool(name="resp", bufs=1))
        sm0 = ctx.enter_context(tc.tile_pool(name="sm0", bufs=1))
        lhs_ctx = ExitStack()
        lhs = lhs_ctx.enter_context(tc.tile_pool(name="lhs", bufs=1))

        with tc.tile_pool(name="prep", bufs=1) as prep, tc.tile_pool(
            name="prep2", bufs=2
        ) as prep2:
            # ---- constants / x ----
            e128 = prep.tile([128, 128], BF16, tag="e128")
            nc.sync.dma_start(e128[:], e128_d)

            # cast + pad d 16->32 (zeros); chunked DMA (a 1MB DMA fans out
            # to too many HWDGE queues for one consumer's sync-wait slots)
            xb = prep.tile([P, I * D2], BF16, tag="xb")
            nc.vector.memset(xb[:], 0.0)
            xbv = xb[:].rearrange("p (i d) -> p i d", i=I)
            for q in range(4):
                xf = prep2.tile([P, ID // 4], F32, tag="xf")
                nc.sync.dma_start(xf[:], x_d[:, q * (ID // 4):(q + 1) * (ID // 4)])
                src = xf[:].rearrange("p (i d) -> p i d", i=I // 4)
                dst = xbv[:, q * (I // 4):(q + 1) * (I // 4), 0:D]
                if q % 2:
                    nc.scalar.copy(dst, src)
                else:
                    nc.vector.tensor_copy(dst, src)

            # ---- Xt: 32 chunks of [(i,d32) rows, p], chunk c at cols 128c ----
            Xt = lhs.tile([128, 32 * 128], BF16)
            for c in range(32):
                pt = pp_t.tile([128, 128], BF16, tag="pt")
                nc.tensor.transpose(pt[:], xb[:, c * 128:(c + 1) * 128], e128[:])
                if c % 2:
                    nc.scalar.copy(Xt[:, c * 128:(c + 1) * 128], pt[:])
                else:
                    nc.vector.tensor_copy(Xt[:, c * 128:(c + 1) * 128], pt[:])

            # ---- W_r: 32 chunks of [(i,d32) rows, (k,o)=512] bf16 ----
            W_r = lhs.tile([128, 32 * KO], BF16)
            # two persistent cast+pad staging buffers; pads zeroed once
            wb0 = prep.tile([K, 4 * D2 * O], BF16, tag="wb0")
            wb1 = prep.tile([K, 4 * D2 * O], BF16, tag="wb1")
            wbs = [wb0, wb1]
            nc.vector.memset(wbs[0][:], 0.0)
            nc.vector.memset(wbs[1][:], 0.0)
            for c in range(32):
                # chunk covers i in [4c, 4c+4): raw [32, 4*D*O=1024] fp32
                wf = prep2.tile([K, 4 * D * O], F32, tag="wf")
                nc.sync.dma_start(wf[:], w_d[:, c * 1024:(c + 1) * 1024])
                wb = wbs[c % 2]
                wdst = wb[:].rearrange("k (i d o) -> k i d o", i=4, d=D2)[
                    :, :, 0:D, :
                ]
                wsrc = wf[:].rearrange("k (i d o) -> k i d o", i=4, d=D)
                if c % 2:
                    nc.scalar.copy(wdst, wsrc)
                else:
                    nc.vector.tensor_copy(wdst, wsrc)
                wv = wb[:].rearrange("k (i d o) -> k i d o", i=4, d=D2)
                pw = pp_w.tile([128, KO], BF16, tag="pw")
                for o0 in range(O):
                    # in: [32, (i4,d32)=128] at fixed o -> out [128, 32]
                    nc.tensor.transpose(
                        pw[:, o0 * 32:(o0 + 1) * 32],
                        wv[:, :, :, o0],
                        e128[0:32, 0:32],
                    )
                # psum cols are (o,k); store as (k,o)
                src = (
                    pw[:]
                    .rearrange("p (o k) -> p o k", o=O, k=K)
                    .transpose([0, 2, 1])
                )
                dst = W_r[:, c * KO:(c + 1) * KO].rearrange(
                    "p (k o) -> p k o", k=K
                )
                if c % 2:
                    nc.scalar.copy(dst, src)
                else:
                    nc.vector.tensor_copy(dst, src)

        # ---- production: s0 (all k) + res (all k), then routing per group ----
        ps0 = pp_s.tile([P, KO], F32, tag="ps0")
        for c in range(32):
            nc.tensor.matmul(
                ps0[:],
                Xt[:, c * 128:(c + 1) * 128],
                W_r[:, c * KO:(c + 1) * KO],
                start=(c == 0),
                stop=(c == 31),
            )
        s0_all = sm0.tile([P, KO], F32)
        nc.scalar.mul(s0_all[:], ps0[:], 1.0 / I)

        res = rp.tile([P, K * I * O], BF16)
        resv = res[:].rearrange("p (k i o) -> p k i o", k=K, i=I, o=O)
        for i in range(I):
            c, r0 = i // 4, (i % 4) * 32
            pr = pp_r.tile([P, KO], F32, tag="pr")  # one full PSUM bank
            nc.tensor.matmul(
                pr[:],
                Xt[r0:r0 + 32, c * 128:(c + 1) * 128],
                W_r[r0:r0 + 32, c * KO:(c + 1) * KO],
                start=True,
                stop=True,
                tile_position=(r0, 0),
            )
            src_ap = pr[:].rearrange("p (k o) -> p k o", k=K)
            if i % 2:
                nc.scalar.copy(resv[:, :, i, :], src_ap)
            else:
                nc.vector.tensor_copy(resv[:, :, i, :], src_ap)

        # ---- routing (Xt/W_r freed) ----
        lhs_ctx.close()
        sm = ctx.enter_context(tc.tile_pool(name="small", bufs=1))
        eps_t = sm.tile([P, 1], F32, tag="eps")
        nc.vector.memset(eps_t[:], EPS)

        def squash(s_ap, v_ap, tag):
            ssq = sm.tile([P, GW], F32, tag="ssq")
            nc.vector.tensor_mul(ssq[:], s_ap, s_ap)
            sq = sm.tile([P, KC], F32, tag=f"sq_{tag}")
            nc.vector.tensor_reduce(
                sq[:], ssq[:].rearrange("p (k o) -> p k o", k=KC), X, ADD
            )
            a = sm.tile([P, KC], F32, tag="sqa")
            nc.scalar.activation(a[:], sq[:], SQRT, bias=eps_t[:])
            b = sm.tile([P, KC], F32, tag="sqb")
            nc.vector.scalar_tensor_tensor(b[:], sq[:], 1.0, a[:], ADD, MULT)
            r = sm.tile([P, KC], F32, tag="sqr")
            nc.vector.reciprocal(r[:], b[:])
            f = sm.tile([P, KC], F32, tag="sqf")
            nc.vector.tensor_mul(f[:], sq[:], r[:])
            nc.vector.tensor_mul(
                v_ap.rearrange("p (k o) -> p k o", k=KC),
                s_ap.rearrange("p (k o) -> p k o", k=KC),
                f[:].unsqueeze(2).broadcast_to([P, KC, O]),
            )

        for g in range(NG):
            rv = resv[:, g * KC:(g + 1) * KC]
            s0 = s0_all[:, g * GW:(g + 1) * GW]

            def uv_pass(vb_t, t_t):
                """t = U v (contract o)."""
                tmp = sm.tile([P, KC * I * O], BF16, tag="tmp")
                tmpv = tmp[:].rearrange("p (k i o) -> p k i o", k=KC, i=I, o=O)
                nc.vector.tensor_mul(
                    tmpv,
                    rv,
                    vb_t[:]
                    .rearrange("p (k o) -> p k o", k=KC)
                    .unsqueeze(2)
                    .broadcast_to([P, KC, I, O]),
                )
                nc.vector.tensor_reduce(
                    t_t[:].rearrange("p (k i) -> p k i", k=KC), tmpv, X, ADD
                )

            def ut_pass(t_t, m_t):
                """m = U^T t (contract i)."""
                tmp = sm.tile([P, KC * I * O], BF16, tag="tmp")
                tmp_kio = (
                    tmp[:]
                    .rearrange("p (k o i) -> p k o i", k=KC, o=O, i=I)
                    .transpose([0, 1, 3, 2])
                )
                nc.vector.tensor_mul(
                    tmp_kio,
                    rv,
                    t_t[:]
                    .rearrange("p (k i) -> p k i", k=KC)
                    .unsqueeze(3)
                    .broadcast_to([P, KC, I, O]),
                )
                tmp_koi = tmp[:].rearrange(
                    "p (k o i) -> p k o i", k=KC, o=O, i=I
                )
                nc.vector.tensor_reduce(
                    m_t[:].rearrange("p (k o) -> p k o", k=KC), tmp_koi, X, ADD
                )

            with nc.allow_low_precision(reason="fp16 routing intermediates"):
                v0 = sm.tile([P, GW], F32, tag="v0")
                squash(s0, v0[:], "v0")
                v0b = sm.tile([P, GW], BF16, tag="v0b")
                nc.vector.tensor_copy(v0b[:], v0[:])

                t_a = sm.tile([P, KC * I], BF16, tag="t")
                uv_pass(v0b, t_a)
                m_a = sm.tile([P, GW], BF16, tag="m")
                ut_pass(t_a, m_a)

                s1 = sm.tile([P, GW], F32, tag="s1")
                nc.vector.tensor_add(s1[:], s0, m_a[:])
                v1 = sm.tile([P, GW], F32, tag="v1")
                squash(s1[:], v1[:], "v1")
                vs = sm.tile([P, GW], F32, tag="vs")
                nc.vector.tensor_add(vs[:], v0[:], v1[:])
                vsb = sm.tile([P, GW], BF16, tag="vsb")
                nc.vector.tensor_copy(vsb[:], vs[:])

                t_b = sm.tile([P, KC * I], BF16, tag="t")
                uv_pass(vsb, t_b)
                m_b = sm.tile([P, GW], BF16, tag="m")
                ut_pass(t_b, m_b)

                s2 = sm.tile([P, GW], F32, tag="s2")
                nc.vector.tensor_add(s2[:], s0, m_b[:])
                outt = sm.tile([P, GW], F32, tag="outt")
                squash(s2[:], outt[:], "out")

            nc.sync.dma_start(out_d[:, g * GW:(g + 1) * GW], outt[:])

    nc.compile()
    return nc


def _get_program():
    global _PROGRAM
    if _PROGRAM is None:
        _PROGRAM = _build_program()
    return _PROGRAM


def _make_in_maps(inputs):
    x = np.ascontiguousarray(np.asarray(inputs["inputs"], dtype=np.float32))
    W = np.ascontiguousarray(np.asarray(inputs["W"], dtype=np.float32))
    assert x.shape == (16, 8, 8, 128, 16) and W.shape == (32, 128, 16, 16)
    xs = x.reshape(N_CORES, P, ID)  # [core, p=128, (i,d)]
    wflat = np.ascontiguousarray(W.reshape(K, I * D * O))
    e128 = np.eye(128, dtype=np.float16)
    return [
        {"x": np.ascontiguousarray(xs[c]), "w": wflat, "e128": e128}
        for c in range(N_CORES)
    ]


def kernel(**inputs):
    from concourse.bass_utils import run_bass_kernel_spmd

    nc = _get_program()
    in_maps = _make_in_maps(inputs)
    r = run_bass_kernel_spmd(nc, in_maps, list(range(N_CORES)))
    outs = [r.results[c]["out"].reshape(2, 8, 8, K, O) for c in range(N_CORES)]
    return np.concatenate(outs, axis=0).astype(np.float32)



# revision 15
# speedup vs baseline: 1.6651x; 1.6651x over previous
"""CapsLayer2D dynamic-routing kernel for 8x TRN2 NeuronCores.

Problem (hardcoded shapes):
  inputs: [B=16, R=8, C=8, I=128, DIN=16] fp32
  W:      [K=32, I=128, DIN=16, DOUT=16] fp32
  out:    [B, R, C, K, DOUT] fp32

Math (3-round dynamic routing, closed form, verified vs reference):
  U[p,k]    = res[p,k,:,:]  (I x O per position p=(b,r,c) and k)
  s0        = mean_i U_i ; v0 = squash(s0)
  t_a = U v0 ; m_a = U^T t_a ; s1 = s0 + m_a ; v1 = squash(s1)
  t_b = U (v0+v1) ; m_b = U^T t_b ; s2 = s0 + m_b ; out = squash(s2)

Sharding: batch across 8 cores (128 positions/core), W replicated.

Performance design (v2):
  - All W/X layout work (pad d 16->32, transpose to matmul operand
    layout, fp32->fp16 cast) is done host-side in numpy: device does
    zero prep, DMAs land operands ready for the PE.
  - Per k-group (4 groups of 8 caps) production: s0 via 32 accumulating
    matmuls, res via 128 per-i matmuls (tile_position quadrants).
    PSUM evictions split Act/Pool so the DVE only routes.
  - Routing avoids tensor_reduce (no DVE perf modes -> 1x) and
    transposed writes (4x slower than 1x). Every big op is a
    scalar_tensor_tensor (4x perf mode: 2-byte dtype, unit-stride
    innermost, SBUF) followed by a log2 tree of stt-adds on sliced
    views, which keeps the innermost axis contiguous for any reduce
    axis. The i-contraction (U^T t) reads t through a duplicated-pair
    tile t2[p,k,i,2] so the broadcast-over-o operand still presents a
    unit-stride innermost [1,2] access pattern.
"""

import sys

import numpy as np

sys.path.insert(0, "/opt/trn_rl_repo")

P, I, D, K, O = 128, 128, 16, 32, 16
D2 = 32  # padded d
ID = I * D  # 2048
KO = K * O  # 512
KC = 8  # k-group size
NG = K // KC  # 4 groups
GW = KC * O  # 128 group output width
GKO = 32 * KC * O  # per-group W_r col count per: 32 chunks x (k8,o16) = 4096
N_CORES = 8
EPS = 1e-7

_PROGRAM = None


def _build_program():
    from contextlib import ExitStack

    import concourse.bass as bass  # noqa: F401
    import concourse.tile as tile
    from concourse import bacc, mybir

    F32 = mybir.dt.float32
    F16 = mybir.dt.float16
    MULT = mybir.AluOpType.mult
    ADD = mybir.AluOpType.add
    X = mybir.AxisListType.X
    SQRT = mybir.ActivationFunctionType.Sqrt

    nc = bacc.Bacc("TRN2", target_bir_lowering=False, debug=False)

    # host-prepped operands (see _make_in_maps):
    #   xt: [(i4,d32) rows, (chunk, p)] fp16  -- lhsT chunks
    #   wr: [(i4,d32) rows, (group, chunk, k8, o)] fp16 -- rhs chunks
    xt_d = nc.dram_tensor("xt", [128, 32 * 128], F16, kind="ExternalInput").ap()
    wr_d = nc.dram_tensor("wr", [128, NG * GKO], F16, kind="ExternalInput").ap()
    out_d = nc.dram_tensor("out", [P, KO], F32, kind="ExternalOutput").ap()

    with ExitStack() as ctx:
        tc = ctx.enter_context(tile.TileContext(nc))

        pp_s = ctx.enter_context(tc.tile_pool(name="pp_s", bufs=2, space="PSUM"))
        pp_r = ctx.enter_context(tc.tile_pool(name="pp_r", bufs=2, space="PSUM"))

        xp = ctx.enter_context(tc.tile_pool(name="xt", bufs=1))
        wp = ctx.enter_context(tc.tile_pool(name="wr", bufs=2))
        rp = ctx.enter_context(tc.tile_pool(name="res", bufs=2))
        sp = ctx.enter_context(tc.tile_pool(name="scratch", bufs=1))
        sm = ctx.enter_context(tc.tile_pool(name="small", bufs=1))

        # ---- inputs ----
        Xt = xp.tile([128, 32 * 128], F16)
        for q in range(4):
            nc.sync.dma_start(
                Xt[:, q * 1024:(q + 1) * 1024],
                xt_d[:, q * 1024:(q + 1) * 1024],
            )

        eps_t = sm.tile([P, 1], F32, tag="eps")
        nc.vector.memset(eps_t[:], EPS)

        def squash(s_ap, v_ap, tag):
            """v = squash(s) for one k-group; s,v fp32 [P, (k8,o16)]."""
            ssq = sm.tile([P, GW], F32, tag=f"ssq_{tag}")
            nc.vector.tensor_mul(ssq[:], s_ap, s_ap)
            sq = sm.tile([P, KC], F32, tag=f"sq_{tag}")
            nc.vector.tensor_reduce(
                sq[:], ssq[:].rearrange("p (k o) -> p k o", k=KC), X, ADD
            )
            a = sm.tile([P, KC], F32, tag=f"sqa_{tag}")
            nc.scalar.activation(a[:], sq[:], SQRT, bias=eps_t[:])
            b = sm.tile([P, KC], F32, tag=f"sqb_{tag}")
            nc.vector.scalar_tensor_tensor(b[:], sq[:], 1.0, a[:], ADD, MULT)
            r = sm.tile([P, KC], F32, tag=f"sqr_{tag}")
            nc.vector.reciprocal(r[:], b[:])
            f = sm.tile([P, KC], F32, tag=f"sqf_{tag}")
            nc.vector.tensor_mul(f[:], sq[:], r[:])
            nc.vector.tensor_mul(
                v_ap.rearrange("p (k o) -> p k o", k=KC),
                s_ap.rearrange("p (k o) -> p k o", k=KC),
                f[:].unsqueeze(2).broadcast_to([P, KC, O]),
            )

        def uv_pass(rv4, vb_t, t2_t, tag):
            """t2[p,k,i,2] = (sum_o res[p,k,i,o] * v[p,k,o]) duplicated.

            rv4: res view [p,k,i,o]; vb_t fp16 [P, GW].
            """
            tmp = sp.tile([P, KC * I * O], F16, tag="s16384")
            t4 = tmp[:].rearrange("p (k i o) -> p k i o", k=KC, i=I)
            vkb = vb_t[:].rearrange("p (k o) -> p k o", k=KC)
            # TENSOR_SCALAR_PTR APs are limited to 2 canonical free dims:
            # one op per k keeps the v broadcast at [i(bcast0), o(unit)]
            for k in range(KC):
                nc.vector.scalar_tensor_tensor(
                    t4[:, k],
                    rv4[:, k],
                    1.0,
                    vkb[:, k].unsqueeze(1).broadcast_to([P, I, O]),
                    MULT,
                    MULT,
                )
            # tree-reduce over o (all stt-adds keep innermost unit stride)
            r8 = sp.tile([P, KC * I * 8], F16, tag="s8192")
            r8v = r8[:].rearrange("p (k i o) -> p k i o", k=KC, i=I)
            nc.vector.scalar_tensor_tensor(
                r8v, t4[:, :, :, 0:8], 1.0, t4[:, :, :, 8:16], MULT, ADD
            )
            r4 = sp.tile([P, KC * I * 4], F16, tag="s4096")
            r4v = r4[:].rearrange("p (k i o) -> p k i o", k=KC, i=I)
            nc.vector.scalar_tensor_tensor(
                r4v, r8v[:, :, :, 0:4], 1.0, r8v[:, :, :, 4:8], MULT, ADD
            )
            r2 = sp.tile([P, KC * I * 2], F16, tag="s2048")
            r2v = r2[:].rearrange("p (k i o) -> p k i o", k=KC, i=I)
            nc.vector.scalar_tensor_tensor(
                r2v, r4v[:, :, :, 0:2], 1.0, r4v[:, :, :, 2:4], MULT, ADD
            )
            # final sum written to both slots of t2 (pair-duplicated t)
            t2v = t2_t[:].rearrange("p (k i two) -> p k i two", k=KC, i=I)
            nc.vector.scalar_tensor_tensor(
                t2v[:, :, :, 0:1], r2v[:, :, :, 0:1], 1.0,
                r2v[:, :, :, 1:2], MULT, ADD,
            )
            nc.vector.scalar_tensor_tensor(
                t2v[:, :, :, 1:2], r2v[:, :, :, 0:1], 1.0,
                r2v[:, :, :, 1:2], MULT, ADD,
            )

        def ut_pass(rv5, t2_t, m_t, tag):
            """m[p,(k,o)] = sum_i res[p,k,i,o] * t[p,k,i].

            Reads t through the duplicated-pair view so the innermost AP
            stays [1,2] (4x mode); reduces over i by tree on sliced views.
            """
            tmp = sp.tile([P, KC * I * O], F16, tag="s16384")
            t5 = tmp[:].rearrange(
                "p (k i o2 two) -> p k i o2 two", k=KC, i=I, o2=O // 2
            )
            t2v = t2_t[:].rearrange("p (k i two) -> p k i two", k=KC, i=I)
            # one op per o-pair: in1 = t2 is then fully contiguous and the
            # sliced out/in0 views canonicalize to [(k i) stride16, pair]
            for j in range(O // 2):
                nc.vector.scalar_tensor_tensor(
                    t5[:, :, :, j], rv5[:, :, :, j], 1.0, t2v, MULT, MULT
                )
            # tree-reduce over i
            cur = tmp[:].rearrange("p (k i o) -> p k i o", k=KC, i=I)
            n = I
            while n > 1:
                h = n // 2
                nxt_t = sp.tile([P, KC * h * O], F16, tag=f"s{KC * h * O}")
                nxt = nxt_t[:].rearrange("p (k i o) -> p k i o", k=KC, i=h)
                nc.vector.scalar_tensor_tensor(
                    nxt, cur[:, :, 0:h, :], 1.0, cur[:, :, h:n, :], MULT, ADD
                )
                cur, n = nxt, h
            nc.vector.tensor_copy(
                m_t[:].rearrange("p (k o) -> p k o", k=KC), cur[:, :, 0, :]
            )

        for g in range(NG):
            # ---- W for this group ----
            W_g = wp.tile([128, GKO], F16, tag="wg")
            for q in range(4):
                nc.sync.dma_start(
                    W_g[:, q * 1024:(q + 1) * 1024],
                    wr_d[:, g * GKO + q * 1024:g * GKO + (q + 1) * 1024],
                )

            # ---- s0 = mean_i u_hat ---- (full PSUM bank tile)
            ps0 = pp_s.tile([P, 512], F32, tag="ps0")
            for c in range(32):
                nc.tensor.matmul(
                    ps0[:, 0:GW],
                    Xt[:, c * 128:(c + 1) * 128],
                    W_g[:, c * 128:(c + 1) * 128],
                    start=(c == 0),
                    stop=(c == 31),
                )
            s0 = sm.tile([P, GW], F32, tag="s0")
            nc.scalar.mul(s0[:], ps0[:, 0:GW], 1.0 / I)

            v0 = sm.tile([P, GW], F32, tag="v0")
            squash(s0[:], v0[:], "v0")
            v0h = sm.tile([P, GW], F16, tag="v0h")
            nc.vector.tensor_copy(v0h[:], v0[:])

            # ---- res (u_hat) for this group: [p, (k8, i, o)] fp16 ----
            # Each matmul gets a full PSUM bank (concurrent start/stop
            # groups must not share a bank); two i's per 2-bank tile,
            # evicted with one strided cross-bank copy on Act.
            res = rp.tile([P, KC * I * O], F16, tag="res")
            resv = res[:].rearrange("p (k i o) -> p k i o", k=KC, i=I, o=O)
            for c in range(32):
                for m in range(2):
                    prb = pp_r.tile([P, 1024], F32, tag="prb")
                    for j in (2 * m, 2 * m + 1):
                        r0 = j * 32
                        nc.tensor.matmul(
                            prb[:, (j % 2) * 512:(j % 2) * 512 + GW],
                            Xt[r0:r0 + 32, c * 128:(c + 1) * 128],
                            W_g[r0:r0 + 32, c * 128:(c + 1) * 128],
                            start=True,
                            stop=True,
                            tile_position=(r0, 0),
                        )
                    src = prb[:].rearrange("p (i x) -> p i x", i=2)[
                        :, :, 0:GW
                    ].rearrange("p i (k o) -> p i k o", k=KC)
                    dst = resv[
                        :, :, 4 * c + 2 * m:4 * c + 2 * m + 2, :
                    ].transpose([0, 2, 1, 3])
                    nc.scalar.copy(dst, src)

            rv4 = res[:].rearrange("p (k i o) -> p k i o", k=KC, i=I)
            rv5 = res[:].rearrange(
                "p (k i o2 two) -> p k i o2 two", k=KC, i=I, o2=O // 2
            )

            # ---- routing ----
            with nc.allow_low_precision(reason="fp16 routing intermediates"):
                t2a = sp.tile([P, KC * I * 2], F16, tag="t2")
                uv_pass(rv4, v0h, t2a, "a")
                m_a = sm.tile([P, GW], F16, tag="m")
                ut_pass(rv5, t2a, m_a, "a")

                s1 = sm.tile([P, GW], F32, tag="s1")
                nc.vector.tensor_add(s1[:], s0[:], m_a[:])
                v1 = sm.tile([P, GW], F32, tag="v1")
                squash(s1[:], v1[:], "v1")
                vs = sm.tile([P, GW], F32, tag="vs")
                nc.vector.tensor_add(vs[:], v0[:], v1[:])
                vsh = sm.tile([P, GW], F16, tag="vsh")
                nc.vector.tensor_copy(vsh[:], vs[:])

                t2b = sp.tile([P, KC * I * 2], F16, tag="t2")
                uv_pass(rv4, vsh, t2b, "b")
                m_b = sm.tile([P, GW], F16, tag="m")
                ut_pass(rv5, t2b, m_b, "b")

                s2 = sm.tile([P, GW], F32, tag="s2")
                nc.vector.tensor_add(s2[:], s0[:], m_b[:])
                outt = sm.tile([P, GW], F32, tag="outt")
                squash(s2[:], outt[:], "out")

            nc.sync.dma_start(out_d[:, g * GW:(g + 1) * GW], outt[:])

    nc.compile()
    return nc


def _get_program():
    global _PROGRAM
    if _PROGRAM is None:
        _PROGRAM = _build_program()
    return _PROGRAM


def _make_in_maps(inputs):
    x = np.ascontiguousarray(np.asarray(inputs["inputs"], dtype=np.float32))
    W = np.ascontiguousarray(np.asarray(inputs["W"], dtype=np.float32))
    assert x.shape == (16, 8, 8, 128, 16) and W.shape == (32, 128, 16, 16)

    # xt rows: (i%4)*32 + d, cols: (i//4)*128 + p  (d padded 16->32)
    xs = x.reshape(N_CORES, P, I, D)  # [core, p, i, d]
    xt = np.zeros((N_CORES, 4, D2, 32, P), np.float32)
    # [core, i4, d, c, p] <- [core, c, i4, d, p]
    xt[:, :, 0:D] = xs.reshape(N_CORES, P, 32, 4, D).transpose(0, 3, 4, 2, 1)
    xt = xt.reshape(N_CORES, 128, 32 * 128).astype(np.float16)

    # wr rows: (i%4)*32 + d, cols: g*4096 + (i//4)*128 + (k%8)*16 + o
    wv = W.reshape(NG, KC, 32, 4, D, O)  # [g, k8, c, i4, d, o]
    wr = np.zeros((4, D2, NG, 32, KC, O), np.float32)  # [i4, d, g, c, k8, o]
    wr[:, 0:D] = wv.transpose(3, 4, 0, 2, 1, 5)
    wr = np.ascontiguousarray(
        wr.reshape(128, NG * GKO).astype(np.float16)
    )

    return [
        {"xt": np.ascontiguousarray(xt[c]), "wr": wr} for c in range(N_CORES)
    ]


def kernel(**inputs):
    from concourse.bass_utils import run_bass_kernel_spmd

    nc = _get_program()
    in_maps = _make_in_maps(inputs)
    r = run_bass_kernel_spmd(nc, in_maps, list(range(N_CORES)))
    outs = [r.results[c]["out"].reshape(2, 8, 8, K, O) for c in range(N_CORES)]
    return np.concatenate(outs, axis=0).astype(np.float32)


# revision 17
# speedup vs baseline: 2.8818x; 1.7308x over previous
"""CapsLayer2D dynamic-routing kernel for 8x TRN2 NeuronCores.

Problem (hardcoded shapes):
  inputs: [B=16, R=8, C=8, I=128, DIN=16] fp32
  W:      [K=32, I=128, DIN=16, DOUT=16] fp32
  out:    [B, R, C, K, DOUT] fp32

Math (3-round dynamic routing, closed form, verified vs reference):
  U[p,k]    = res[p,k,:,:]  (I x O per position p=(b,r,c) and k)
  s0        = mean_i U_i ; v0 = squash(s0)
  t_a = U v0 ; m_a = U^T t_a ; s1 = s0 + m_a ; v1 = squash(s1)
  t_b = U (v0+v1) ; m_b = U^T t_b ; s2 = s0 + m_b ; out = squash(s2)

Sharding: batch across 8 cores (128 positions/core), W replicated.

Performance design (v2):
  - All W/X layout work (pad d 16->32, transpose to matmul operand
    layout, fp32->fp16 cast) is done host-side in numpy: device does
    zero prep, DMAs land operands ready for the PE.
  - Per k-group (4 groups of 8 caps) production: s0 via 32 accumulating
    matmuls, res via 128 per-i matmuls (tile_position quadrants).
    PSUM evictions split Act/Pool so the DVE only routes.
  - Routing avoids tensor_reduce (no DVE perf modes -> 1x) and
    transposed writes (4x slower than 1x). Every big op is a
    scalar_tensor_tensor (4x perf mode: 2-byte dtype, unit-stride
    innermost, SBUF) followed by a log2 tree of stt-adds on sliced
    views, which keeps the innermost axis contiguous for any reduce
    axis. The i-contraction (U^T t) reads t through a duplicated-pair
    tile t2[p,k,i,2] so the broadcast-over-o operand still presents a
    unit-stride innermost [1,2] access pattern.
"""

import sys

import numpy as np

sys.path.insert(0, "/opt/trn_rl_repo")

P, I, D, K, O = 128, 128, 16, 32, 16
D2 = 32  # padded d
ID = I * D  # 2048
KO = K * O  # 512
KC = 8  # k-group size
NG = K // KC  # 4 groups
GW = KC * O  # 128 group output width
GKO = 32 * KC * O  # per-group W_r col count per: 32 chunks x (k8,o16) = 4096
N_CORES = 8
EPS = 1e-7

_PROGRAM = None


def _build_program():
    from contextlib import ExitStack

    import concourse.bass as bass  # noqa: F401
    import concourse.tile as tile
    from concourse import bacc, mybir

    F32 = mybir.dt.float32
    F16 = mybir.dt.float16
    MULT = mybir.AluOpType.mult
    ADD = mybir.AluOpType.add
    X = mybir.AxisListType.X
    SQRT = mybir.ActivationFunctionType.Sqrt

    nc = bacc.Bacc("TRN2", target_bir_lowering=False, debug=False)

    # host-prepped operands (see _make_in_maps):
    #   xt: [(i4,d32) rows, (chunk, p)] fp16  -- lhsT chunks
    #   wr: [(i4,d32) rows, (group, chunk, k8, o)] fp16 -- rhs chunks
    xt_d = nc.dram_tensor("xt", [128, 32 * 128], F16, kind="ExternalInput").ap()
    wr_d = nc.dram_tensor("wr", [128, NG * GKO], F16, kind="ExternalInput").ap()
    out_d = nc.dram_tensor("out", [P, KO], F32, kind="ExternalOutput").ap()

    with ExitStack() as ctx:
        tc = ctx.enter_context(tile.TileContext(nc))

        pp_s = ctx.enter_context(tc.tile_pool(name="pp_s", bufs=2, space="PSUM"))
        pp_r = ctx.enter_context(tc.tile_pool(name="pp_r", bufs=2, space="PSUM"))

        xp = ctx.enter_context(tc.tile_pool(name="xt", bufs=1))
        wp = ctx.enter_context(tc.tile_pool(name="wr", bufs=2))
        rp = ctx.enter_context(tc.tile_pool(name="res", bufs=2))
        sp = ctx.enter_context(tc.tile_pool(name="scratch", bufs=1))
        sm = ctx.enter_context(tc.tile_pool(name="small", bufs=1))

        # ---- inputs ----
        Xt = xp.tile([128, 32 * 128], F16)
        for q in range(4):
            nc.sync.dma_start(
                Xt[:, q * 1024:(q + 1) * 1024],
                xt_d[:, q * 1024:(q + 1) * 1024],
            )

        eps_t = sm.tile([P, 1], F32, tag="eps")
        nc.vector.memset(eps_t[:], EPS)

        def squash(s_ap, v_ap, tag):
            """v = squash(s) for one k-group; s,v fp32 [P, (k8,o16)]."""
            ssq = sm.tile([P, GW], F32, tag=f"ssq_{tag}")
            nc.vector.tensor_mul(ssq[:], s_ap, s_ap)
            sq = sm.tile([P, KC], F32, tag=f"sq_{tag}")
            nc.vector.tensor_reduce(
                sq[:], ssq[:].rearrange("p (k o) -> p k o", k=KC), X, ADD
            )
            a = sm.tile([P, KC], F32, tag=f"sqa_{tag}")
            nc.scalar.activation(a[:], sq[:], SQRT, bias=eps_t[:])
            b = sm.tile([P, KC], F32, tag=f"sqb_{tag}")
            nc.vector.scalar_tensor_tensor(b[:], sq[:], 1.0, a[:], ADD, MULT)
            r = sm.tile([P, KC], F32, tag=f"sqr_{tag}")
            nc.vector.reciprocal(r[:], b[:])
            f = sm.tile([P, KC], F32, tag=f"sqf_{tag}")
            nc.vector.tensor_mul(f[:], sq[:], r[:])
            nc.vector.tensor_mul(
                v_ap.rearrange("p (k o) -> p k o", k=KC),
                s_ap.rearrange("p (k o) -> p k o", k=KC),
                f[:].unsqueeze(2).broadcast_to([P, KC, O]),
            )

        def uv_pass(rv4, vb_t, t2_t, tag):
            """t2[p,k,i,2] = (sum_o res[p,k,i,o] * v[p,k,o]) duplicated.

            rv4: res view [p,k,i,o]; vb_t fp16 [P, GW].
            All big ops are TENSOR_TENSOR (2x mode: fp16, unit-stride
            innermost; two-stream ops have no 4x on TRN2).
            """
            tmp = sp.tile([P, KC * I * O], F16, tag="s16384")
            t4 = tmp[:].rearrange("p (k i o) -> p k i o", k=KC, i=I)
            vb4 = (
                vb_t[:]
                .rearrange("p (k o) -> p k o", k=KC)
                .unsqueeze(2)
                .broadcast_to([P, KC, I, O])
            )
            nc.vector.tensor_mul(t4, rv4, vb4)
            # tree-reduce over o (slice halves keep innermost unit stride)
            r8 = sp.tile([P, KC * I * 8], F16, tag="s8192")
            r8v = r8[:].rearrange("p (k i o) -> p k i o", k=KC, i=I)
            nc.vector.tensor_add(r8v, t4[:, :, :, 0:8], t4[:, :, :, 8:16])
            r4 = sp.tile([P, KC * I * 4], F16, tag="s4096")
            r4v = r4[:].rearrange("p (k i o) -> p k i o", k=KC, i=I)
            nc.vector.tensor_add(r4v, r8v[:, :, :, 0:4], r8v[:, :, :, 4:8])
            r2 = sp.tile([P, KC * I * 2], F16, tag="s2048")
            r2v = r2[:].rearrange("p (k i o) -> p k i o", k=KC, i=I)
            nc.vector.tensor_add(r2v, r4v[:, :, :, 0:2], r4v[:, :, :, 2:4])
            # final sum written to both slots of t2 (pair-duplicated t)
            t2v = t2_t[:].rearrange("p (k i two) -> p k i two", k=KC, i=I)
            nc.vector.tensor_add(
                t2v[:, :, :, 0:1], r2v[:, :, :, 0:1], r2v[:, :, :, 1:2]
            )
            nc.vector.tensor_add(
                t2v[:, :, :, 1:2], r2v[:, :, :, 0:1], r2v[:, :, :, 1:2]
            )

        def ut_pass(rv5, t2_t, m_t, tag):
            """m[p,(k,o)] = sum_i res[p,k,i,o] * t[p,k,i].

            Reads t through the duplicated-pair view so the innermost AP
            stays [1,2] (4x mode); reduces over i by tree on sliced views.
            """
            tmp = sp.tile([P, KC * I * O], F16, tag="s16384")
            t5 = tmp[:].rearrange(
                "p (k i o2 two) -> p k i o2 two", k=KC, i=I, o2=O // 2
            )
            tb5 = (
                t2_t[:]
                .rearrange("p (k i two) -> p k i two", k=KC, i=I)
                .unsqueeze(3)
                .broadcast_to([P, KC, I, O // 2, 2])
            )
            nc.vector.tensor_mul(t5, rv5, tb5)
            # tree-reduce over i; last step writes m directly
            cur = tmp[:].rearrange("p (k i o) -> p k i o", k=KC, i=I)
            n = I
            while n > 2:
                h = n // 2
                nxt_t = sp.tile([P, KC * h * O], F16, tag=f"s{KC * h * O}")
                nxt = nxt_t[:].rearrange("p (k i o) -> p k i o", k=KC, i=h)
                nc.vector.tensor_add(nxt, cur[:, :, 0:h, :], cur[:, :, h:n, :])
                cur, n = nxt, h
            nc.vector.tensor_add(
                m_t[:].rearrange("p (k o) -> p k o", k=KC),
                cur[:, :, 0, :],
                cur[:, :, 1, :],
            )

        for g in range(NG):
            # ---- W for this group ----
            W_g = wp.tile([128, GKO], F16, tag="wg")
            for q in range(4):
                nc.sync.dma_start(
                    W_g[:, q * 1024:(q + 1) * 1024],
                    wr_d[:, g * GKO + q * 1024:g * GKO + (q + 1) * 1024],
                )

            # ---- s0 = mean_i u_hat ---- (full PSUM bank tile)
            ps0 = pp_s.tile([P, 512], F32, tag="ps0")
            for c in range(32):
                nc.tensor.matmul(
                    ps0[:, 0:GW],
                    Xt[:, c * 128:(c + 1) * 128],
                    W_g[:, c * 128:(c + 1) * 128],
                    start=(c == 0),
                    stop=(c == 31),
                )
            s0 = sm.tile([P, GW], F32, tag="s0")
            nc.scalar.mul(s0[:], ps0[:, 0:GW], 1.0 / I)

            v0 = sm.tile([P, GW], F32, tag="v0")
            squash(s0[:], v0[:], "v0")
            v0h = sm.tile([P, GW], F16, tag="v0h")
            nc.vector.tensor_copy(v0h[:], v0[:])

            # ---- res (u_hat) for this group: [p, (k8, i, o)] fp16 ----
            # Each matmul gets a full PSUM bank (concurrent start/stop
            # groups must not share a bank); two i's per 2-bank tile,
            # evicted with one strided cross-bank copy on Act.
            res = rp.tile([P, KC * I * O], F16, tag="res")
            resv = res[:].rearrange("p (k i o) -> p k i o", k=KC, i=I, o=O)
            for c in range(32):
                for m in range(2):
                    prb = pp_r.tile([P, 1024], F32, tag="prb")
                    for j in (2 * m, 2 * m + 1):
                        r0 = j * 32
                        nc.tensor.matmul(
                            prb[:, (j % 2) * 512:(j % 2) * 512 + GW],
                            Xt[r0:r0 + 32, c * 128:(c + 1) * 128],
                            W_g[r0:r0 + 32, c * 128:(c + 1) * 128],
                            start=True,
                            stop=True,
                            tile_position=(r0, 0),
                        )
                    src = prb[:].rearrange("p (i x) -> p i x", i=2)[
                        :, :, 0:GW
                    ].rearrange("p i (k o) -> p i k o", k=KC)
                    dst = resv[
                        :, :, 4 * c + 2 * m:4 * c + 2 * m + 2, :
                    ].transpose([0, 2, 1, 3])
                    nc.scalar.copy(dst, src)

            rv4 = res[:].rearrange("p (k i o) -> p k i o", k=KC, i=I)
            rv5 = res[:].rearrange(
                "p (k i o2 two) -> p k i o2 two", k=KC, i=I, o2=O // 2
            )

            # ---- routing ----
            with nc.allow_low_precision(reason="fp16 routing intermediates"):
                t2a = sp.tile([P, KC * I * 2], F16, tag="t2")
                uv_pass(rv4, v0h, t2a, "a")
                m_a = sm.tile([P, GW], F16, tag="m")
                ut_pass(rv5, t2a, m_a, "a")

                s1 = sm.tile([P, GW], F32, tag="s1")
                nc.vector.tensor_add(s1[:], s0[:], m_a[:])
                v1 = sm.tile([P, GW], F32, tag="v1")
                squash(s1[:], v1[:], "v1")
                vs = sm.tile([P, GW], F32, tag="vs")
                nc.vector.tensor_add(vs[:], v0[:], v1[:])
                vsh = sm.tile([P, GW], F16, tag="vsh")
                nc.vector.tensor_copy(vsh[:], vs[:])

                t2b = sp.tile([P, KC * I * 2], F16, tag="t2")
                uv_pass(rv4, vsh, t2b, "b")
                m_b = sm.tile([P, GW], F16, tag="m")
                ut_pass(rv5, t2b, m_b, "b")

                s2 = sm.tile([P, GW], F32, tag="s2")
                nc.vector.tensor_add(s2[:], s0[:], m_b[:])
                outt = sm.tile([P, GW], F32, tag="outt")
                squash(s2[:], outt[:], "out")

            nc.sync.dma_start(out_d[:, g * GW:(g + 1) * GW], outt[:])

    nc.compile()
    return nc


def _get_program():
    global _PROGRAM
    if _PROGRAM is None:
        _PROGRAM = _build_program()
    return _PROGRAM


def _make_in_maps(inputs):
    x = np.ascontiguousarray(np.asarray(inputs["inputs"], dtype=np.float32))
    W = np.ascontiguousarray(np.asarray(inputs["W"], dtype=np.float32))
    assert x.shape == (16, 8, 8, 128, 16) and W.shape == (32, 128, 16, 16)

    # xt rows: (i%4)*32 + d, cols: (i//4)*128 + p  (d padded 16->32)
    xs = x.reshape(N_CORES, P, I, D)  # [core, p, i, d]
    xt = np.zeros((N_CORES, 4, D2, 32, P), np.float32)
    # [core, i4, d, c, p] <- [core, c, i4, d, p]
    xt[:, :, 0:D] = xs.reshape(N_CORES, P, 32, 4, D).transpose(0, 3, 4, 2, 1)
    xt = xt.reshape(N_CORES, 128, 32 * 128).astype(np.float16)

    # wr rows: (i%4)*32 + d, cols: g*4096 + (i//4)*128 + (k%8)*16 + o
    wv = W.reshape(NG, KC, 32, 4, D, O)  # [g, k8, c, i4, d, o]
    wr = np.zeros((4, D2, NG, 32, KC, O), np.float32)  # [i4, d, g, c, k8, o]
    wr[:, 0:D] = wv.transpose(3, 4, 0, 2, 1, 5)
    wr = np.ascontiguousarray(
        wr.reshape(128, NG * GKO).astype(np.float16)
    )

    return [
        {"xt": np.ascontiguousarray(xt[c]), "wr": wr} for c in range(N_CORES)
    ]


def kernel(**inputs):
    from concourse.bass_utils import run_bass_kernel_spmd

    nc = _get_program()
    in_maps = _make_in_maps(inputs)
    r = run_bass_kernel_spmd(nc, in_maps, list(range(N_CORES)))
    outs = [r.results[c]["out"].reshape(2, 8, 8, K, O) for c in range(N_CORES)]
    return np.concatenate(outs, axis=0).astype(np.float32)


# revision 19
# speedup vs baseline: 2.9560x; 1.0258x over previous
"""CapsLayer2D dynamic-routing kernel for 8x TRN2 NeuronCores.

Problem (hardcoded shapes):
  inputs: [B=16, R=8, C=8, I=128, DIN=16] fp32
  W:      [K=32, I=128, DIN=16, DOUT=16] fp32
  out:    [B, R, C, K, DOUT] fp32

Math (3-round dynamic routing, closed form, verified vs reference):
  U[p,k]    = res[p,k,:,:]  (I x O per position p=(b,r,c) and k)
  s0        = mean_i U_i ; v0 = squash(s0)
  t_a = U v0 ; m_a = U^T t_a ; s1 = s0 + m_a ; v1 = squash(s1)
  t_b = U (v0+v1) ; m_b = U^T t_b ; s2 = s0 + m_b ; out = squash(s2)

Sharding: batch across 8 cores (128 positions/core), W replicated.

Performance design (v2):
  - All W/X layout work (pad d 16->32, transpose to matmul operand
    layout, fp32->fp16 cast) is done host-side in numpy: device does
    zero prep, DMAs land operands ready for the PE.
  - Per k-group (4 groups of 8 caps) production: s0 via 32 accumulating
    matmuls, res via 128 per-i matmuls (tile_position quadrants).
    PSUM evictions split Act/Pool so the DVE only routes.
  - Routing avoids tensor_reduce (no DVE perf modes -> 1x) and
    transposed writes (4x slower than 1x). Every big op is a
    scalar_tensor_tensor (4x perf mode: 2-byte dtype, unit-stride
    innermost, SBUF) followed by a log2 tree of stt-adds on sliced
    views, which keeps the innermost axis contiguous for any reduce
    axis. The i-contraction (U^T t) reads t through a duplicated-pair
    tile t2[p,k,i,2] so the broadcast-over-o operand still presents a
    unit-stride innermost [1,2] access pattern.
"""

import sys

import numpy as np

sys.path.insert(0, "/opt/trn_rl_repo")

P, I, D, K, O = 128, 128, 16, 32, 16
D2 = 32  # padded d
ID = I * D  # 2048
KO = K * O  # 512
KC = 8  # k-group size
NG = K // KC  # 4 groups
GW = KC * O  # 128 group output width
GKO = 32 * KC * O  # per-group W_r col count per: 32 chunks x (k8,o16) = 4096
N_CORES = 8
EPS = 1e-7

_PROGRAM = None


def _build_program():
    from contextlib import ExitStack

    import concourse.bass as bass  # noqa: F401
    import concourse.tile as tile
    from concourse import bacc, mybir

    F32 = mybir.dt.float32
    F16 = mybir.dt.float16
    MULT = mybir.AluOpType.mult
    ADD = mybir.AluOpType.add
    X = mybir.AxisListType.X
    SQRT = mybir.ActivationFunctionType.Sqrt

    nc = bacc.Bacc("TRN2", target_bir_lowering=False, debug=False)

    # host-prepped operands (see _make_in_maps):
    #   xt: [(i4,d32) rows, (chunk, p)] fp16  -- lhsT chunks
    #   wr: [(i4,d32) rows, (group, chunk, k8, o)] fp16 -- rhs chunks
    xt_d = nc.dram_tensor("xt", [128, 32 * 128], F16, kind="ExternalInput").ap()
    wr_d = nc.dram_tensor("wr", [128, NG * GKO], F16, kind="ExternalInput").ap()
    out_d = nc.dram_tensor("out", [P, KO], F32, kind="ExternalOutput").ap()

    with ExitStack() as ctx:
        tc = ctx.enter_context(tile.TileContext(nc))

        pp_s = ctx.enter_context(tc.tile_pool(name="pp_s", bufs=2, space="PSUM"))
        pp_r = ctx.enter_context(tc.tile_pool(name="pp_r", bufs=2, space="PSUM"))

        xp = ctx.enter_context(tc.tile_pool(name="xt", bufs=1))
        wp = ctx.enter_context(tc.tile_pool(name="wr", bufs=2))
        rp = ctx.enter_context(tc.tile_pool(name="res", bufs=2))
        sp = ctx.enter_context(tc.tile_pool(name="scratch", bufs=1))
        sm = ctx.enter_context(tc.tile_pool(name="small", bufs=1))

        # ---- inputs ----
        Xt = xp.tile([128, 32 * 128], F16)
        for q in range(4):
            nc.sync.dma_start(
                Xt[:, q * 1024:(q + 1) * 1024],
                xt_d[:, q * 1024:(q + 1) * 1024],
            )

        eps_t = sm.tile([P, 1], F32, tag="eps")
        nc.vector.memset(eps_t[:], EPS)

        def squash(s_ap, v_ap, tag):
            """v = squash(s) for one k-group; s,v fp32 [P, (k8,o16)]."""
            ssq = sm.tile([P, GW], F32, tag=f"ssq_{tag}")
            nc.vector.tensor_mul(ssq[:], s_ap, s_ap)
            sq = sm.tile([P, KC], F32, tag=f"sq_{tag}")
            nc.vector.tensor_reduce(
                sq[:], ssq[:].rearrange("p (k o) -> p k o", k=KC), X, ADD
            )
            a = sm.tile([P, KC], F32, tag=f"sqa_{tag}")
            nc.scalar.activation(a[:], sq[:], SQRT, bias=eps_t[:])
            b = sm.tile([P, KC], F32, tag=f"sqb_{tag}")
            nc.vector.scalar_tensor_tensor(b[:], sq[:], 1.0, a[:], ADD, MULT)
            r = sm.tile([P, KC], F32, tag=f"sqr_{tag}")
            nc.vector.reciprocal(r[:], b[:])
            f = sm.tile([P, KC], F32, tag=f"sqf_{tag}")
            nc.vector.tensor_mul(f[:], sq[:], r[:])
            nc.vector.tensor_mul(
                v_ap.rearrange("p (k o) -> p k o", k=KC),
                s_ap.rearrange("p (k o) -> p k o", k=KC),
                f[:].unsqueeze(2).broadcast_to([P, KC, O]),
            )

        def uv_pass(rv4, vb_t, t2_t, tag):
            """t2[p,k,i,2] = (sum_o res[p,k,i,o] * v[p,k,o]) duplicated.

            rv4: res view [p,k,i,o]; vb_t fp16 [P, GW].
            All big ops are TENSOR_TENSOR (2x mode: fp16, unit-stride
            innermost; two-stream ops have no 4x on TRN2).
            """
            tmp = sp.tile([P, KC * I * O], F16, tag="s16384")
            t4 = tmp[:].rearrange("p (k i o) -> p k i o", k=KC, i=I)
            vb4 = (
                vb_t[:]
                .rearrange("p (k o) -> p k o", k=KC)
                .unsqueeze(2)
                .broadcast_to([P, KC, I, O])
            )
            nc.vector.tensor_mul(t4, rv4, vb4)
            # tree-reduce over o (slice halves keep innermost unit stride)
            r8 = sp.tile([P, KC * I * 8], F16, tag="s8192")
            r8v = r8[:].rearrange("p (k i o) -> p k i o", k=KC, i=I)
            nc.vector.tensor_add(r8v, t4[:, :, :, 0:8], t4[:, :, :, 8:16])
            r4 = sp.tile([P, KC * I * 4], F16, tag="s4096")
            r4v = r4[:].rearrange("p (k i o) -> p k i o", k=KC, i=I)
            nc.vector.tensor_add(r4v, r8v[:, :, :, 0:4], r8v[:, :, :, 4:8])
            r2 = sp.tile([P, KC * I * 2], F16, tag="s2048")
            r2v = r2[:].rearrange("p (k i o) -> p k i o", k=KC, i=I)
            nc.vector.tensor_add(r2v, r4v[:, :, :, 0:2], r4v[:, :, :, 2:4])
            # butterfly: both t2 slots get the pair sum in one 2x op
            # (the reversed-stride operand keeps innermost packed, +-1)
            t2v = t2_t[:].rearrange("p (k i two) -> p k i two", k=KC, i=I)
            nc.vector.tensor_add(t2v, r2v, r2v[:, :, :, ::-1])

        def ut_pass(rv5, t2_t, m_t, tag):
            """m[p,(k,o)] = sum_i res[p,k,i,o] * t[p,k,i].

            Reads t through the duplicated-pair view so the innermost AP
            stays [1,2] (4x mode); reduces over i by tree on sliced views.
            """
            tmp = sp.tile([P, KC * I * O], F16, tag="s16384")
            t5 = tmp[:].rearrange(
                "p (k i o2 two) -> p k i o2 two", k=KC, i=I, o2=O // 2
            )
            tb5 = (
                t2_t[:]
                .rearrange("p (k i two) -> p k i two", k=KC, i=I)
                .unsqueeze(3)
                .broadcast_to([P, KC, I, O // 2, 2])
            )
            nc.vector.tensor_mul(t5, rv5, tb5)
            # tree-reduce over i; last step writes m directly
            cur = tmp[:].rearrange("p (k i o) -> p k i o", k=KC, i=I)
            n = I
            while n > 2:
                h = n // 2
                nxt_t = sp.tile([P, KC * h * O], F16, tag=f"s{KC * h * O}")
                nxt = nxt_t[:].rearrange("p (k i o) -> p k i o", k=KC, i=h)
                nc.vector.tensor_add(nxt, cur[:, :, 0:h, :], cur[:, :, h:n, :])
                cur, n = nxt, h
            nc.vector.tensor_add(
                m_t[:].rearrange("p (k o) -> p k o", k=KC),
                cur[:, :, 0, :],
                cur[:, :, 1, :],
            )

        for g in range(NG):
            # ---- W for this group ----
            W_g = wp.tile([128, GKO], F16, tag="wg")
            for q in range(4):
                nc.sync.dma_start(
                    W_g[:, q * 1024:(q + 1) * 1024],
                    wr_d[:, g * GKO + q * 1024:g * GKO + (q + 1) * 1024],
                )

            # ---- s0 = mean_i u_hat ---- (full PSUM bank tile)
            ps0 = pp_s.tile([P, 512], F32, tag="ps0")
            for c in range(32):
                nc.tensor.matmul(
                    ps0[:, 0:GW],
                    Xt[:, c * 128:(c + 1) * 128],
                    W_g[:, c * 128:(c + 1) * 128],
                    start=(c == 0),
                    stop=(c == 31),
                )
            s0 = sm.tile([P, GW], F32, tag="s0")
            nc.scalar.mul(s0[:], ps0[:, 0:GW], 1.0 / I)

            v0 = sm.tile([P, GW], F32, tag="v0")
            squash(s0[:], v0[:], "v0")
            v0h = sm.tile([P, GW], F16, tag="v0h")
            nc.vector.tensor_copy(v0h[:], v0[:])

            # ---- res (u_hat) for this group: [p, (k8, i, o)] fp16 ----
            # Each matmul gets a full PSUM bank (concurrent start/stop
            # groups must not share a bank); two i's per 2-bank tile,
            # evicted with one strided cross-bank copy on Act.
            res = rp.tile([P, KC * I * O], F16, tag="res")
            resv = res[:].rearrange("p (k i o) -> p k i o", k=KC, i=I, o=O)
            for c in range(32):
                for m in range(2):
                    prb = pp_r.tile([P, 1024], F32, tag="prb")
                    for j in (2 * m, 2 * m + 1):
                        r0 = j * 32
                        nc.tensor.matmul(
                            prb[:, (j % 2) * 512:(j % 2) * 512 + GW],
                            Xt[r0:r0 + 32, c * 128:(c + 1) * 128],
                            W_g[r0:r0 + 32, c * 128:(c + 1) * 128],
                            start=True,
                            stop=True,
                            tile_position=(r0, 0),
                        )
                    src = prb[:].rearrange("p (i x) -> p i x", i=2)[
                        :, :, 0:GW
                    ].rearrange("p i (k o) -> p i k o", k=KC)
                    dst = resv[
                        :, :, 4 * c + 2 * m:4 * c + 2 * m + 2, :
                    ].transpose([0, 2, 1, 3])
                    # group 0: DVE is idle before its res exists, so split
                    # the eviction load to cut the startup critical path
                    if g == 0 and m == 1:
                        nc.vector.tensor_copy(dst, src)
                    else:
                        nc.scalar.copy(dst, src)

            rv4 = res[:].rearrange("p (k i o) -> p k i o", k=KC, i=I)
            rv5 = res[:].rearrange(
                "p (k i o2 two) -> p k i o2 two", k=KC, i=I, o2=O // 2
            )

            # ---- routing ----
            with nc.allow_low_precision(reason="fp16 routing intermediates"):
                t2a = sp.tile([P, KC * I * 2], F16, tag="t2")
                uv_pass(rv4, v0h, t2a, "a")
                m_a = sm.tile([P, GW], F16, tag="m")
                ut_pass(rv5, t2a, m_a, "a")

                s1 = sm.tile([P, GW], F32, tag="s1")
                nc.vector.tensor_add(s1[:], s0[:], m_a[:])
                v1 = sm.tile([P, GW], F32, tag="v1")
                squash(s1[:], v1[:], "v1")
                vs = sm.tile([P, GW], F32, tag="vs")
                nc.vector.tensor_add(vs[:], v0[:], v1[:])
                vsh = sm.tile([P, GW], F16, tag="vsh")
                nc.vector.tensor_copy(vsh[:], vs[:])

                t2b = sp.tile([P, KC * I * 2], F16, tag="t2")
                uv_pass(rv4, vsh, t2b, "b")
                m_b = sm.tile([P, GW], F16, tag="m")
                ut_pass(rv5, t2b, m_b, "b")

                s2 = sm.tile([P, GW], F32, tag="s2")
                nc.vector.tensor_add(s2[:], s0[:], m_b[:])
                outt = sm.tile([P, GW], F32, tag="outt")
                squash(s2[:], outt[:], "out")

            nc.sync.dma_start(out_d[:, g * GW:(g + 1) * GW], outt[:])

    nc.compile()
    return nc


def _get_program():
    global _PROGRAM
    if _PROGRAM is None:
        _PROGRAM = _build_program()
    return _PROGRAM


def _make_in_maps(inputs):
    x = np.ascontiguousarray(np.asarray(inputs["inputs"], dtype=np.float32))
    W = np.ascontiguousarray(np.asarray(inputs["W"], dtype=np.float32))
    assert x.shape == (16, 8, 8, 128, 16) and W.shape == (32, 128, 16, 16)

    # xt rows: (i%4)*32 + d, cols: (i//4)*128 + p  (d padded 16->32)
    xs = x.reshape(N_CORES, P, I, D)  # [core, p, i, d]
    xt = np.zeros((N_CORES, 4, D2, 32, P), np.float32)
    # [core, i4, d, c, p] <- [core, c, i4, d, p]
    xt[:, :, 0:D] = xs.reshape(N_CORES, P, 32, 4, D).transpose(0, 3, 4, 2, 1)
    xt = xt.reshape(N_CORES, 128, 32 * 128).astype(np.float16)

    # wr rows: (i%4)*32 + d, cols: g*4096 + (i//4)*128 + (k%8)*16 + o
    wv = W.reshape(NG, KC, 32, 4, D, O)  # [g, k8, c, i4, d, o]
    wr = np.zeros((4, D2, NG, 32, KC, O), np.float32)  # [i4, d, g, c, k8, o]
    wr[:, 0:D] = wv.transpose(3, 4, 0, 2, 1, 5)
    wr = np.ascontiguousarray(
        wr.reshape(128, NG * GKO).astype(np.float16)
    )

    return [
        {"xt": np.ascontiguousarray(xt[c]), "wr": wr} for c in range(N_CORES)
    ]


def kernel(**inputs):
    from concourse.bass_utils import run_bass_kernel_spmd

    nc = _get_program()
    in_maps = _make_in_maps(inputs)
    r = run_bass_kernel_spmd(nc, in_maps, list(range(N_CORES)))
    outs = [r.results[c]["out"].reshape(2, 8, 8, K, O) for c in range(N_CORES)]
    return np.concatenate(outs, axis=0).astype(np.float32)
